# revision 7
# baseline (speedup 1.0000x reference)
"""Trainium2 Bass kernel for nn_CascadeGradNetOURS (dense_mlp) — v2.

Math (reference):
    h = x @ W.T                       # (B, E), shared by all layers
    z = beta[0]*(h + b[0])
    for i in 0..6:
        z = beta[i+1]*(h + b[i+1]) + alpha[i]*relu(z)
    z = alpha[7]*relu(z)
    out = z @ W + bias_last           # (B, IN)

Device formulation (batch-sharded 1024 rows/core, transposed layout
hT[e, b]; per-layer scalars become per-partition vectors):

Sign-deferred z-tracking: X_k = sigma_k * z_k, sigma_1 = 1,
sigma_{k+1} = sign(alpha[k-1]).  With A_k = sigma_{k+1}*beta[k],
B_k = A_k*biases[k], S_k = sigma_k*|alpha[k-1]|:

    X_2     = (A_1*h + B_1) + relu((S_1*A_0)*h + S_1*B_0)       [one fused DVE op]
    X_{k+1} = (A_k*h + B_k) + relu(S_k*X_k)      k=2..7          [hbb + fused DVE op]
    z_out   = max(S_8*X_8, 0)                                    [tensor_scalar]
    out     = z_out @ (sign(alpha[7])*W) + bias_last             [mm2, sign folded]

The fused op is a custom DVE instruction CASCADE_FUSE_ANT:
    out = in0 + relu(in1*s0 + s1)
with a hand-authored 2x_1p micro-op program (fp16, 2 elem/cycle).

mm2 runs in 3 windows (ec 0..19 staged via ACT + bias_last fold during
phase A, ec 20..29 accumulated in-place via DVE, ec 30..31 combined to
fp32 + DMA).
"""

import os

os.environ.setdefault("MYCRO_LOCAL_CACHE", "1")

import numpy as np

import concourse.bacc as bacc
import concourse.bass as bass
import concourse.mybir as mybir
from concourse.tile import TileContext

N_CORES = 8
B, IN, E, L = 8192, 1024, 4096, 8
BC = B // N_CORES          # 1024 batch rows per core
NI = IN // 128             # 8 i-chunks
NE = E // 128              # 32 e-chunks
F16 = mybir.dt.float16
F32 = mybir.dt.float32
NCONST = 26

GROUP = 4
ACT_HBB = (1, 3, 5, 7)     # hbb layers computed on the scalar engine
W1E = 20                   # mm2 window-1: ec 0..19 (staged via ACT during phase A)
W2BE = 30                  # (unused) merged window-2: ec 20..31 single combine
W1_SCHED = {5: range(0, 6), 6: range(6, 12), 7: range(12, 16)}


# ---------------------------------------------------------------------------
# Custom DVE op: out = in0 + relu(in1*s0 + s1), with 2x_1p microcode.
# ---------------------------------------------------------------------------
_CASC_OP = None


def _register_cascade_fuse():
    global _CASC_OP
    if _CASC_OP is not None:
        return _CASC_OP
    import copy

    from concourse import dve_ops
    from concourse.dve_spec import Spec, Src0, Src1, C0, C1, relu, lower
    from concourse.dve_uop import (
        DveOpSpec, UopDpConfig, InpSel, OutSel, OutPath, AluInp, DelayInp, AluOp,
    )

    for op in dve_ops.OPS:
        if op.name == "CASCADE_FUSE_ANT":
            _CASC_OP = op
            return op

    spec = Spec(
        body=Src0 + relu(Src1 * C0 + C1),
        reference=lambda in0, in1, s0, s1, imm2: (
            in0.astype(np.float32)
            + np.maximum(np.nan_to_num(in1.astype(np.float32) * s0 + s1), 0)
        ),
    )
    uops_1x = lower(spec, ver="v3")
    assert len(uops_1x) == 1

    # 2x_1p: lane0=SRC_1 (b0 ALU direct), chains c0=SRC_0, c1=CONST_0,
    # c2=CONST_1, c3=ZERO, c4=SRC_0_HI, c5=SRC_1_HI; c5 then relays the
    # lo/hi intermediates down the pipe.
    u = copy.deepcopy(uops_1x[0])
    u.inp = [InpSel.ZERO] * len(u.inp)
    u.inp_enable = [0] * len(u.inp_enable)
    u.enable_input(InpSel.SRC_1, 0)
    u.enable_input(InpSel.SRC_0, 1)
    u.enable_input(InpSel.CONST_0, 2)
    u.enable_input(InpSel.CONST_1, 3)
    u.enable_input(InpSel.ZERO, 4)
    u.enable_input(InpSel.SRC_0_HI, 5)
    u.enable_input(InpSel.SRC_1_HI, 6)
    dp = [UopDpConfig() for _ in range(8)]
    # b0: m_lo = SRC_1 * C0; load all chains from input lanes
    dp[0].enable_alu(AluOp.MULTIPLY, AluInp.PREV_ALU_OUT, AluInp.PREV_DELAY_1)
    for c in range(6):
        dp[0].enable_delay_from_src(DelayInp.PREV_DELAY, c)
    # b1: m_hi = SRC_1_HI * C0 ; c5 <- m_lo ; pass c0,c2,c3,c4
    dp[1].enable_alu(AluOp.MULTIPLY, AluInp.PREV_DELAY_5, AluInp.PREV_DELAY_1)
    dp[1].enable_delay_from_src(DelayInp.PREV_ALU_OUT, 5)
    dp[1].pass_through_delay(0, 2, 3, 4)
    # b2: ba_lo = m_lo + C1 ; c5 <- m_hi ; pass c0,c2,c3,c4
    dp[2].enable_alu(AluOp.ADD, AluInp.PREV_DELAY_5, AluInp.PREV_DELAY_2)
    dp[2].enable_delay_from_src(DelayInp.PREV_ALU_OUT, 5)
    dp[2].pass_through_delay(0, 2, 3, 4)
    # b3: ba_hi = m_hi + C1 ; c5 <- ba_lo ; pass c0,c3,c4
    dp[3].enable_alu(AluOp.ADD, AluInp.PREV_DELAY_5, AluInp.PREV_DELAY_2)
    dp[3].enable_delay_from_src(DelayInp.PREV_ALU_OUT, 5)
    dp[3].pass_through_delay(0, 3, 4)
    # b4: r_lo = max(ba_lo, 0) ; c5 <- ba_hi ; pass c0,c3,c4
    dp[4].enable_alu(AluOp.MAX, AluInp.PREV_DELAY_5, AluInp.PREV_DELAY_3)
    dp[4].enable_delay_from_src(DelayInp.PREV_ALU_OUT, 5)
    dp[4].pass_through_delay(0, 3, 4)
    # b5: r_hi = max(ba_hi, 0) ; c5 <- r_lo ; pass c0,c4
    dp[5].enable_alu(AluOp.MAX, AluInp.PREV_DELAY_5, AluInp.PREV_DELAY_3)
    dp[5].enable_delay_from_src(DelayInp.PREV_ALU_OUT, 5)
    dp[5].pass_through_delay(0, 4)
    # b6: o_lo = SRC_0 + r_lo ; c5 <- r_hi ; pass c4
    dp[6].enable_alu(AluOp.ADD, AluInp.PREV_DELAY_0, AluInp.PREV_DELAY_5)
    dp[6].enable_delay_from_src(DelayInp.PREV_ALU_OUT, 5)
    dp[6].pass_through_delay(4)
    # b7: o_hi = SRC_0_HI + r_hi ; c0 <- o_lo
    dp[7].enable_alu(AluOp.ADD, AluInp.PREV_DELAY_4, AluInp.PREV_DELAY_5)
    dp[7].enable_delay_from_src(DelayInp.PREV_ALU_OUT, 0)
    u.datapath_config = dp
    u.out = {p: OutSel.ALU_OUT for p in OutPath}
    u.out_enable = {p: 0 for p in OutPath}
    u.enable_output(OutSel.DELAY_0, OutPath.WR0_LO)   # o_lo
    u.enable_output(OutSel.ALU_OUT, OutPath.WR0_HI)   # o_hi
    uops_2x = [u]

    row = 1 + len(dve_ops.OPS)
    assert row < 0x20

    class FusedDveOp(dve_ops.DveOp):
        def compile(self, ver):
            key = (self.name, ver)
            cached = dve_ops._COMPILE_CACHE.get(key)
            if cached is not None:
                return cached
            assert ver == "v3", f"CASCADE_FUSE_ANT authored for v3 only, got {ver}"
            result = DveOpSpec(
                name=self.name, opcode=row, uops=uops_1x, uops_2x=uops_2x,
                rd1_en=True, perf_max=1,
            )
            dve_ops._COMPILE_CACHE[key] = result
            return result

    op = FusedDveOp("CASCADE_FUSE_ANT", spec, subdim=False, uops_sha={})
    dve_ops.OPS.append(op)
    dve_ops._SUB_OPCODE_FOR_NAME[op.name] = row
    dve_ops.CUSTOM_DVE_SPECS[op.name] = op.spec
    _CASC_OP = op
    return op


def build_nc() -> bass.Bass:
    nc = bacc.Bacc()
    AL = mybir.AluOpType
    AF = mybir.ActivationFunctionType
    op = _register_cascade_fuse()

    xTd = nc.declare_dram_parameter("xT", [128, NI, BC], F16, isOutput=False)
    WTd = nc.declare_dram_parameter("WT", [128, NE, NI, 128], F16, isOutput=False)
    W2d = nc.declare_dram_parameter("W2", [128, NE, IN], F16, isOutput=False)
    Cd = nc.declare_dram_parameter("consts", [128, NE, NCONST], F32, isOutput=False)
    Bd = nc.declare_dram_parameter("blast", [128, NI], F32, isOutput=False)
    Od = nc.declare_dram_parameter("outT", [128, NI, BC], F32, isOutput=True)

    def casc(out, in0, in1, s0, s1):
        return nc.vector._custom_dve(op, out=out, in0=in0, in1=in1, s0=s0, s1=s1)

    with TileContext(nc) as tc:
        with (
            tc.tile_pool(name="persist", bufs=1) as persist,
            tc.tile_pool(name="wtp", bufs=2) as wtp,
            tc.tile_pool(name="hsbp", bufs=5) as hsbp,
            tc.tile_pool(name="hbbp", bufs=6) as hbbp,
            tc.tile_pool(name="xp", bufs=6) as xp,
            tc.tile_pool(name="outp", bufs=2) as outp,
            tc.tile_pool(name="psum_h", bufs=2, space="PSUM") as psum_h,
            tc.tile_pool(name="psum_o", bufs=3, space="PSUM") as psum_o,
        ):
            wt_pref = {}
            for ec in range(2):
                wt = wtp.tile([128, NI, 128], F16, tag="wt")
                nc.sync.dma_start(out=wt, in_=WTd[:, ec, :, :])
                wt_pref[ec] = wt
            x_sb = []
            for i in range(NI):
                t = persist.tile([128, BC], F16, name=f"x_{i}")
                nc.sync.dma_start(out=t, in_=xTd[:, i, :])
                x_sb.append(t)
            consts_sb = persist.tile([128, NE, NCONST], F32)
            nc.sync.dma_start(out=consts_sb, in_=Cd[:, :, :])
            blast_sb = persist.tile([128, NI], F32)
            nc.sync.dma_start(out=blast_sb, in_=Bd[:, :])
            w2_sb = persist.tile([128, NE, IN], F16)
            z_sb = persist.tile([128, NE, BC], F16)
            o_acc = persist.tile([128, NI, BC], F16)

            def c_ap(ec, col):
                return consts_sb[:, ec, col : col + 1]

            def emit_og_pass(og, ec_lo, ec_hi, mode):
                ic, hf = og // 2, og % 2
                bsl = hf * 512
                ops = psum_o.tile([128, 512], F32, tag="o", name=f"o_{og}_{ec_lo}")
                for ec in range(ec_lo, ec_hi):
                    nc.tensor.matmul(
                        ops,
                        w2_sb[:, ec, ic * 128 : (ic + 1) * 128],
                        z_sb[:, ec, bsl : bsl + 512],
                        start=(ec == ec_lo),
                        stop=(ec == ec_hi - 1),
                    )
                osl = o_acc[:, ic, bsl : bsl + 512]
                if mode == 1:     # stage + bias_last fold (ACT)
                    nc.scalar.activation(
                        out=osl, in_=ops, func=AF.Identity,
                        bias=blast_sb[:, ic : ic + 1], scale=1.0,
                    )
                elif mode == 2:   # accumulate in place (DVE)
                    nc.vector.tensor_tensor(out=osl, in0=osl, in1=ops, op=AL.add)
                else:             # final combine -> fp32 -> DRAM
                    osb = outp.tile([128, 512], F32, tag="osb")
                    nc.vector.tensor_tensor(out=osb, in0=osl, in1=ops, op=AL.add)
                    nc.scalar.dma_start(out=Od[:, ic, bsl : bsl + 512], in_=osb)

            # ---------------- Phase A: mm1 + cascade ----------------
            for g in range(NE // GROUP):
                g0 = g * GROUP
                if g == 1:
                    # w2 needed from the first W1 og pass (~mid phase A)
                    for q in range(4):
                        nc.sync.dma_start(
                            out=w2_sb[:, q * 8 : (q + 1) * 8, :],
                            in_=W2d[:, q * 8 : (q + 1) * 8, :],
                        )
                ecs = range(g0, g0 + GROUP)
                hp = {}
                for ec in ecs:
                    if ec in wt_pref:
                        wt = wt_pref.pop(ec)
                    else:
                        wt = wtp.tile([128, NI, 128], F16, tag="wt")
                        nc.sync.dma_start(out=wt, in_=WTd[:, ec, :, :])
                    ps = psum_h.tile([128, BC], F32, tag="h")
                    for i in range(NI):
                        lhsT = wt[:, i, :]
                        for hf in range(2):
                            nc.tensor.matmul(
                                ps[:, hf * 512 : (hf + 1) * 512],
                                lhsT,
                                x_sb[i][:, hf * 512 : (hf + 1) * 512],
                                start=(i == 0),
                                stop=(i == NI - 1),
                            )
                    hp[ec] = ps
                # cascade, layer-major across the group; ACT-side hbb tiles
                # are emitted one layer ahead so the DVE chain never waits
                hsb = {}
                X = {}
                hbbq = {}

                def one_act_hbb(ec, k):
                    t = hbbp.tile([128, BC], F16, tag="hbb", name=f"hbb_{ec}_{k}")
                    nc.scalar.activation(
                        out=t, in_=hsb[ec], func=AF.Identity,
                        bias=c_ap(ec, 8 + k), scale=c_ap(ec, k),
                    )
                    hbbq[(ec, k)] = t

                def emit_act_hbb(k):
                    for ec in ecs:
                        one_act_hbb(ec, k)

                for ec in ecs:
                    t = hsbp.tile([128, BC], F16, tag="hsb", name=f"hsb_{ec}")
                    nc.scalar.copy(out=t, in_=hp[ec])
                    hsb[ec] = t
                    one_act_hbb(ec, 1)
                for k in range(1, L):
                    if k + 1 < L and (k + 1) in ACT_HBB:
                        emit_act_hbb(k + 1)
                    for ec in ecs:
                        if k in ACT_HBB:
                            hbb = hbbq.pop((ec, k))
                        else:
                            hbb = hbbp.tile([128, BC], F16, tag="hbb", name=f"hbb_{ec}_{k}")
                            nc.vector.tensor_scalar(
                                hbb, hsb[ec], c_ap(ec, k), c_ap(ec, 8 + k),
                                AL.mult, AL.add,
                            )
                        xo = xp.tile([128, BC], F16, tag="x", name=f"x_{ec}_{k}")
                        if k == 1:
                            casc(xo, hbb, hsb[ec], c_ap(ec, 24), c_ap(ec, 25))
                        else:
                            casc(xo, hbb, X[ec], c_ap(ec, 15 + k), 0.0)
                        X[ec] = xo
                for ec in ecs:
                    nc.vector.tensor_scalar(
                        z_sb[:, ec, :], X[ec], c_ap(ec, 23), 0.0, AL.mult, AL.max
                    )
                for p in W1_SCHED.get(g, ()):
                    emit_og_pass(p, 0, W1E, mode=1)

            # ---------------- Phase B: mm2 windows 2+3 ----------------
            for og in range(16):
                emit_og_pass(og, W1E, NE, mode=3)

    # enable the 2x_1p perf-mode path on every fused-cascade instruction
    # (the BassInstruction wrapper returned at emit time is not the module
    # object, so this must be set on the serialized instructions directly)
    for bb in nc.m.functions[0].blocks:
        for i in bb.instructions:
            if i.__class__.__name__ == "InstCustomDveAnt":
                i.perf_max = 1

    nc.compile()
    return nc


def _prep_inputs(x, W, biases, bias_last, alpha, beta):
    """Host-side shard/relayout/constant precompute. Returns per-core in_maps."""
    x = np.asarray(x, np.float32)
    W = np.asarray(W, np.float32)
    biases = np.asarray(biases, np.float32)
    bias_last = np.asarray(bias_last, np.float32)
    alpha = np.asarray(alpha, np.float32)
    beta = np.asarray(beta, np.float32)

    sgn = lambda a: np.where(a >= 0, 1.0, -1.0).astype(np.float32)
    sigma = np.ones((L + 1, E), np.float32)
    for k in range(1, L):
        sigma[k + 1] = sgn(alpha[k - 1])
    s_last = sgn(alpha[L - 1])

    consts = np.zeros((E, NCONST), np.float32)
    A = np.zeros((L, E), np.float32)
    Bb = np.zeros((L, E), np.float32)
    S = np.zeros((L, E), np.float32)
    for k in range(L):
        A[k] = sigma[k + 1] * beta[k]
        Bb[k] = A[k] * biases[k]
    for k in range(1, L + 1):
        S[k - 1] = sigma[k] * np.abs(alpha[k - 1])
    consts[:, 0:8] = A.T
    consts[:, 8:16] = Bb.T
    consts[:, 16:24] = S.T
    consts[:, 24] = S[0] * A[0]
    consts[:, 25] = S[0] * Bb[0]
    consts_t = np.ascontiguousarray(
        consts.reshape(NE, 128, NCONST).transpose(1, 0, 2)
    )

    WT_t = np.ascontiguousarray(
        W.T.reshape(NI, 128, NE, 128).transpose(1, 2, 0, 3).astype(np.float16)
    )
    W2 = W * s_last[:, None]
    W2_t = np.ascontiguousarray(
        W2.reshape(NE, 128, IN).transpose(1, 0, 2).astype(np.float16)
    )
    blast_t = np.ascontiguousarray(bias_last.reshape(NI, 128).T)

    in_maps = []
    for c in range(N_CORES):
        xc = x[c * BC : (c + 1) * BC]
        xT = np.ascontiguousarray(
            xc.T.reshape(NI, 128, BC).transpose(1, 0, 2).astype(np.float16)
        )
        in_maps.append(
            {"xT": xT, "WT": WT_t, "W2": W2_t, "consts": consts_t, "blast": blast_t}
        )
    return in_maps


_NC_CACHE = None


def _install_ntff_hook():
    import sys
    import types

    if "antenv.axon_hooks" in sys.modules:
        return
    try:
        from trn_agent_boot.trn_boot import _ntff_profile_via_ctypes

        hook = _ntff_profile_via_ctypes("/opt/axon/libaxon_pjrt.so")
    except Exception:
        hook = None
    m = types.ModuleType("antenv.axon_hooks")
    m.get_axon_ntff_profile_hook = lambda: hook
    m.set_axon_ntff_profile_hook = lambda h: None
    sys.modules["antenv.axon_hooks"] = m


def run(inputs: dict, trace: bool = False):
    global _NC_CACHE
    from concourse.bass_utils import run_bass_kernel_spmd

    if trace:
        _install_ntff_hook()

    if _NC_CACHE is None:
        _NC_CACHE = build_nc()
    nc = _NC_CACHE
    in_maps = _prep_inputs(**inputs)
    res = run_bass_kernel_spmd(nc, in_maps, list(range(N_CORES)), trace=trace)
    out = np.empty((B, IN), np.float32)
    for c in range(N_CORES):
        oc = np.asarray(res.results[c]["outT"])
        out_core = oc.transpose(1, 0, 2).reshape(IN, BC)
        out[c * BC : (c + 1) * BC] = out_core.T
    return out, res


def kernel(x, W, biases, bias_last, alpha, beta) -> np.ndarray:
    out, _ = run(
        dict(x=x, W=W, biases=biases, bias_last=bias_last, alpha=alpha, beta=beta)
    )
    return out
